# revision 17
# baseline (speedup 1.0000x reference)
"""CTRNN policy kernel for Trainium2 (8 NeuronCores, batch-parallel).

Reference computation (per batch element b, B=64, N=1024, OBS=64, A=16):
    I = E[b] @ obs[b]
    repeat ITERS x:  y = tanh(gain*(v+bias))*mask
                     v = (v + DT/tau * (-v + W[b]@y + I)) * mask
    action[b] = D[b] @ v

Sharding: batch 64 -> 8 cores x 8 individuals, fully data parallel.

Algebraic refactor (all folds on host):
    am = DT/tau*mask, cm = (1-DT/tau)*mask
    s  = g*(v+bias)                  (state; g = gain, zero-guarded)
    Wg = diag(g*am) W diag(mask)     -> bf16 on device (SBUF-resident)
    Ig = g*(am*(E@obs) + bias*(1-cm))
    per iteration: y = tanh(s);  s' = cm*s + Wg@y + Ig
    action = (D/g) @ s_final - D@bias

Per-core schedule: 2 rounds x 4 individuals, ROUND-MAJOR: all 9
iterations of round 0 run first, then all of round 1. W arrives on a
single consumption-ordered DMA queue (round 0's 8MB first, ~64 tiles of
[128,1024] bf16), so round 0 computes at full speed from ~1/2 of the
DMA time while round 1's W streams in behind it; round 1 then runs
without DMA stalls. The aggregate per-core DMA rate (~270 GB/s with 8
cores active, less under SBUF contention) makes W delivery the
dominant cost, so this overlap is the main lever.

Per iteration-round unit: the matvec for the 4 individuals runs on the
4 PE column strips (tile_position col-tiling): stationary = y column
[128,1] bf16, moving = Wg^T slab [128,512] bf16, all 4 strips stream
concurrently (~206ns per (chunk,half) group), outputs land as rows
[1,512] at PSUM partitions {0,32,64,96}. VectorE reads each full PSUM
bank fusing the +Ig add into a bf16 tile (dead lanes free), PE
transposes [128,128] bf16 blocks back to column layout (bf16 identity:
1 cyc/row), and per half a single batched [128,16] add + tanh updates
all 4 individuals' states (column order: col = 4*chunk + q), so the
next unit's first matvecs only wait on the first half's tanh.

Decode runs on the PE column strips per round (round 0's decode hides
in round 1's DMA tail); output is [128, 2]: partition 32q+a = action
dim a of individual q, column = round.
"""

import os
import sys
from contextlib import ExitStack

import numpy as np

for _p in ("/opt/trn_rl_repo", "/root/.axon_site/_ro/trn_rl_repo"):
    if os.path.isdir(_p) and _p not in sys.path:
        sys.path.append(_p)

import concourse.bass as bass  # noqa: E402
import concourse.tile as tile  # noqa: E402
from concourse import bacc, mybir  # noqa: E402
from concourse.bass_utils import run_bass_kernel_spmd  # noqa: E402

DT = 0.1
ITERS = int(1.0 // DT)  # == 9: reference.py uses `int(1.0 // DT)`, and 1.0//0.1 == 9.0
B_FULL, N, OBS, ADIM = 64, 1024, 64, 16
NCORES = 8
BPC = B_FULL // NCORES  # individuals per core
P = 128
NCH = 8                 # 128-chunks per vector
RQ = 4                  # individuals per round (one per PE column strip)
NR = 2                  # rounds
F32 = mybir.dt.float32
BF16 = mybir.dt.bfloat16


def make_pools(ctx, tc):
    return dict(
        const=ctx.enter_context(tc.tile_pool(name="const", bufs=1)),
        wp1=ctx.enter_context(tc.tile_pool(name="w1", bufs=BPC * NCH)),
        state=ctx.enter_context(tc.tile_pool(name="state", bufs=2)),
        prow=ctx.enter_context(tc.tile_pool(name="prow", bufs=6, space="PSUM")),
        ptr=ctx.enter_context(tc.tile_pool(name="ptr", bufs=2, space="PSUM")),
    )


# W tile groups: (start chunk, n chunks). Small first for a fast matmul
# start, large for DMA throughput; single queue keeps arrival order equal
# to consumption order.
WGROUPS = [(h, 1) for h in range(NCH)]


def load_w(nc, wpools, ins):
    """Load W as consumption-ordered tiles on the sync queue.

    Returns {(b, h): (tile, col offset of chunk h)}.
    """
    w_sb = {}
    for r in range(NR):
        for h0, nch in WGROUPS:
            for q in range(RQ):
                b = RQ * r + q
                wt = wpools[f"wp{nch}"].tile(
                    [P, nch * N], BF16, tag=f"w{nch}", name=f"w{b}g{h0}")
                nc.sync.dma_start(
                    wt[:], ins["Wsb"][b][:, h0 * N:(h0 + nch) * N])
                for h in range(h0, h0 + nch):
                    w_sb[b, h] = (wt, (h - h0) * N)
    return w_sb


def kernel_body(ctx, tc, ins, out_ap, iters=ITERS, pools=None, w_sb=None):
    nc = tc.nc
    Tanh = mybir.ActivationFunctionType.Tanh
    add = mybir.AluOpType.add
    mult = mybir.AluOpType.mult
    sub = mybir.AluOpType.subtract

    p = pools if pools is not None else make_pools(ctx, tc)
    const, state = p["const"], p["state"]
    prow, ptr = p["prow"], p["ptr"]

    # ---- constants; s0c rides ahead of W on the sync queue ----
    s0_sb = const.tile([P, NR * 32], F32, tag="s0", name="s0")
    nc.sync.dma_start(s0_sb[:], ins["s0c"][:])
    cm_sb = const.tile([P, NR * 32], F32, tag="cm", name="cm")
    nc.gpsimd.dma_start(cm_sb[:], ins["cmc"][:])
    ident_sb = const.tile([P, P], BF16, tag="ident", name="ident")
    nc.gpsimd.dma_start(ident_sb[:], ins["ident"][:])
    igc_sb = const.tile([P, NR * 32], F32, tag="igc", name="igc")
    nc.gpsimd.dma_start(igc_sb[:], ins["igc"][:])

    # ---- initial state + host-computed y0 per round ----
    y0_sb = const.tile([P, NR * 32], BF16, tag="y0c", name="y0c")
    nc.sync.dma_start(y0_sb[:], ins["y0c"][:])
    s_cur = [s0_sb[:, 32 * r:32 * r + 32] for r in range(NR)]
    y_cur = [y0_sb[:, 32 * r:32 * r + 32] for r in range(NR)]

    # ---- W loads: consumption-ordered tiles on the sync queue ----
    if w_sb is None:
        w_sb = load_w(nc, p, ins)

    # ---- decode constants (needed late; tail of gpsimd queue) ----
    dgt_sb = const.tile([P, BPC * P], F32, tag="dgt", name="dgt")
    nc.gpsimd.dma_start(dgt_sb[:], ins["dgtc"][:])
    db0_sb = const.tile([P, NR], F32, tag="db0", name="db0")
    nc.gpsimd.dma_start(db0_sb[:], ins["db0"][:])
    act_sb = const.tile([P, NR], F32, tag="act", name="act")

    pd_tile = [None]

    def decode_round(r):
        # action strip-matmuls for round r; borrows a prow bank
        if pd_tile[0] is None:
            pd_tile[0] = prow.tile([P, 512], F32, tag="pr", name="pd")
        pd = pd_tile[0]
        for q in range(RQ):
            b = RQ * r + q
            for h in range(NCH):
                nc.tensor.matmul(
                    pd[32 * q:32 * q + ADIM, r:r + 1],
                    dgt_sb[:, P * b + ADIM * h: P * b + ADIM * h + ADIM],
                    s_cur[r][:, 4 * h + q: 4 * h + q + 1],
                    start=(h == 0), stop=(h == NCH - 1),
                    tile_position=(0, 32 * q),
                )

    # ---- recurrent loop: all of round 0's iterations, then round 1's.
    # Round r's W arrives first (single queue, round-major), so round 0
    # runs at full speed from ~1/2 DMA time while round 1's W streams in;
    # round 1 starts right when its W lands. State updates split in half
    # so the next unit's first matvecs never wait on the full tanh. ----
    for r in range(NR):
        for t in range(iters):
            tmp = state.tile([P, 32], F32, tag=f"t{r}", name=f"t{r}")
            nc.gpsimd.tensor_tensor(
                tmp[:], cm_sb[:, 32 * r:32 * r + 32], s_cur[r][:], op=mult)
            tmp2 = state.tile([P, 32], F32, tag=f"t2{r}", name=f"t2{r}")
            nc.gpsimd.tensor_tensor(
                tmp2[:], tmp[:], igc_sb[:, 32 * r:32 * r + 32], op=add)
            pr = [prow.tile([P, 512], F32, tag="pr", name="pr")
                  for _ in range(2)]
            for j in range(2):
                for h in range(NCH):
                    for q in range(RQ):
                        b = RQ * r + q
                        wt, woff = w_sb[b, h]
                        nc.tensor.matmul(
                            pr[j][32 * q:32 * q + 1, :],
                            y_cur[r][:, 4 * h + q: 4 * h + q + 1],
                            wt[:, woff + 512 * j:woff + 512 * j + 512],
                            start=(h == 0), stop=(h == NCH - 1),
                            tile_position=(0, 32 * q),
                        )
            # PSUM -> bf16 SBUF converts (Ig is added on the small side):
            # u0 whole on DVE (ready mid-unit); u1 split across DVE + Act so
            # neither blocks the s'a -> tanh-a chain that gates the next unit
            u0 = state.tile([P, 512], BF16, tag=f"u{r}0", name=f"u{r}0")
            nc.vector.tensor_scalar_add(u0[:], pr[0][:], 0.0)
            u1a = state.tile([P, 256], BF16, tag=f"u{r}1a", name=f"u{r}1a")
            nc.vector.tensor_scalar_add(u1a[:], pr[1][:, 0:256], 0.0)
            u1b = state.tile([P, 256], BF16, tag=f"u{r}1b", name=f"u{r}1b")
            nc.scalar.activation(u1b[:], pr[1][:, 256:512],
                                 mybir.ActivationFunctionType.Copy)
            usrc = [(u0, 0), (u0, 128), (u0, 256), (u0, 384),
                    (u1a, 0), (u1a, 128), (u1b, 0), (u1b, 128)]
            pt = ptr.tile([P, NCH, P], BF16, tag="pt", name="pt")
            s_n = state.tile([P, 32], F32, tag=f"s{r}", name=f"s{r}")
            y_n = (state.tile([P, 32], BF16, tag=f"y{r}", name=f"y{r}")
                   if t < iters - 1 else None)
            for half in range(2):
                for t8 in range(4 * half, 4 * half + 4):
                    ut, uo = usrc[t8]
                    nc.tensor.transpose(
                        pt[:, t8, :], ut[:, uo:uo + 128], ident_sb[:])
                ha, hb = 16 * half, 16 * half + 16
                nc.vector.tensor_tensor(
                    s_n[:, ha:hb], tmp2[:, ha:hb],
                    pt[:, 4 * half:4 * half + 4, 0:97:32], op=add)
                if y_n is not None:
                    nc.scalar.activation(
                        y_n[:, ha:hb], s_n[:, ha:hb], Tanh)
            s_cur[r] = s_n
            if y_n is not None:
                y_cur[r] = y_n
        decode_round(r)

    # ---- final: action = pd - D@bias ----
    nc.vector.tensor_tensor(act_sb[:], pd_tile[0][:, 0:NR], db0_sb[:], op=sub)
    nc.sync.dma_start(out_ap[:], act_sb[:])
    pd_tile[0] = None


def build_nc(iters=ITERS, reps=1, w_once=False):
    nc = bacc.Bacc(
        "TRN2", target_bir_lowering=False, debug=False, enable_asserts=False,
    )
    ins = {}
    for name, shape, dt in [
        ("Wsb", [BPC, P, NCH * N], BF16),
        ("s0c", [P, NR * 32], F32),
        ("y0c", [P, NR * 32], BF16),
        ("cmc", [P, NR * 32], F32),
        ("igc", [P, NR * 32], F32),
        ("dgtc", [P, BPC * P], F32),
        ("db0", [P, NR], F32),
        ("ident", [P, P], BF16),
    ]:
        ins[name] = nc.dram_tensor(name, shape, dt, kind="ExternalInput").ap()
    out_ap = nc.dram_tensor("act", [P, NR], F32, kind="ExternalOutput").ap()

    with tile.TileContext(nc) as tc:
        with ExitStack() as ctx:
            pools = make_pools(ctx, tc)
            w_shared = None
            if w_once:
                nc0 = tc.nc
                w_shared = load_w(nc0, pools, ins)
            for _rep in range(reps):
                kernel_body(ctx, tc, ins, out_ap, iters, pools,
                            w_sb=w_shared)
    nc.compile()
    return nc


def prep_in_maps(obs, v0, tau, gain, bias, W, mask, E, D):
    f = np.float32
    obs, v0, tau, gain, bias, W, mask, E, D = [
        np.asarray(x, dtype=f) for x in (obs, v0, tau, gain, bias, W, mask, E, D)
    ]
    import ml_dtypes
    bf16 = ml_dtypes.bfloat16

    g = np.where(gain == 0.0, f(1e-6), gain)    # exact-rescaling guard
    am = (DT / tau) * mask                      # [64, N]
    cm = (1.0 - DT / tau) * mask
    I = np.einsum("bno,bo->bn", E, obs)         # [64, N]
    Ig = g * (am * I + bias * (1.0 - cm))
    s0 = g * (v0 + bias)
    Wg = W * (g * am)[:, :, None] * mask[:, None, :]
    # device layout: w[b][k, h*N + m] = Wg[b, m, 128h+k]
    wdev = np.ascontiguousarray(
        Wg.transpose(0, 2, 1).reshape(B_FULL, NCH, P, N).transpose(0, 2, 1, 3)
    ).reshape(B_FULL, P, NCH * N).astype(bf16)

    def cols(x):  # [64, N] -> [core, p, 32r + 4t + q]  (n = p + 128 t)
        xc = x.reshape(NCORES, NR, RQ, NCH, P)
        return np.ascontiguousarray(
            xc.transpose(0, 4, 1, 3, 2)).reshape(NCORES, P, NR * 32)

    s0c = cols(s0)
    y0c = cols(np.tanh(s0)).astype(bf16)
    cmc = cols(cm)
    igc = cols(Ig)
    Dg = D / g[:, None, :]
    dgt = np.ascontiguousarray(
        Dg.transpose(0, 2, 1).reshape(B_FULL, NCH, P, ADIM).transpose(0, 2, 1, 3)
    ).reshape(B_FULL, P, P)
    dgtc = np.ascontiguousarray(
        dgt.reshape(NCORES, BPC, P, P).transpose(0, 2, 1, 3)
    ).reshape(NCORES, P, BPC * P)
    db0 = np.einsum("ban,bn->ba", D, bias)      # [64, ADIM]
    # db0c[core][32q + a, r] = db0[8core + 4r + q, a]
    db0c = np.zeros((NCORES, P, NR), f)
    for r in range(NR):
        for q in range(RQ):
            db0c[:, 32 * q:32 * q + ADIM, r] = db0.reshape(
                NCORES, NR, RQ, ADIM)[:, r, q]
    ident = np.eye(P, dtype=f).astype(bf16)

    in_maps = []
    for core in range(NCORES):
        s = slice(core * BPC, (core + 1) * BPC)
        in_maps.append({
            "Wsb": np.ascontiguousarray(wdev[s]),
            "s0c": s0c[core], "y0c": y0c[core],
            "cmc": cmc[core], "igc": igc[core],
            "dgtc": dgtc[core], "db0": db0c[core], "ident": ident,
        })
    return in_maps


_NC_CACHE = None


def _get_nc():
    global _NC_CACHE
    if _NC_CACHE is None:
        _NC_CACHE = build_nc()
    return _NC_CACHE


def kernel(obs, v0, tau, gain, bias, W, mask, E, D):
    nc = _get_nc()
    in_maps = prep_in_maps(obs, v0, tau, gain, bias, W, mask, E, D)
    res = run_bass_kernel_spmd(nc, in_maps, core_ids=list(range(NCORES)))
    # device output is [128, NR] per core: [32q + a, r] = action[4r+q, a]
    out = np.empty((B_FULL, ADIM), np.float32)
    for core in range(NCORES):
        a = np.asarray(res.results[core]["act"])
        for r in range(NR):
            for q in range(RQ):
                out[core * BPC + RQ * r + q] = a[32 * q:32 * q + ADIM, r]
    return out


# revision 18
# speedup vs baseline: 1.0485x; 1.0485x over previous
"""CTRNN policy kernel for Trainium2 (8 NeuronCores, batch-parallel).

Reference computation (per batch element b, B=64, N=1024, OBS=64, A=16):
    I = E[b] @ obs[b]
    repeat ITERS x:  y = tanh(gain*(v+bias))*mask
                     v = (v + DT/tau * (-v + W[b]@y + I)) * mask
    action[b] = D[b] @ v

Sharding: batch 64 -> 8 cores x 8 individuals, fully data parallel.

Algebraic refactor (all folds on host):
    am = DT/tau*mask, cm = (1-DT/tau)*mask
    s  = g*(v+bias)                  (state; g = gain, zero-guarded)
    Wg = diag(g*am) W diag(mask)     -> bf16 on device (SBUF-resident)
    Ig = g*(am*(E@obs) + bias*(1-cm))
    per iteration: y = tanh(s);  s' = cm*s + Wg@y + Ig
    action = (D/g) @ s_final - D@bias

Per-core schedule: 2 rounds x 4 individuals, ROUND-MAJOR: all 9
iterations of round 0 run first, then all of round 1. W arrives on a
single consumption-ordered DMA queue (round 0's 8MB first, ~64 tiles of
[128,1024] bf16), so round 0 computes at full speed from ~1/2 of the
DMA time while round 1's W streams in behind it; round 1 then runs
without DMA stalls. The aggregate per-core DMA rate (~270 GB/s with 8
cores active, less under SBUF contention) makes W delivery the
dominant cost, so this overlap is the main lever.

Per iteration-round unit: the matvec for the 4 individuals runs on the
4 PE column strips (tile_position col-tiling): stationary = y column
[128,1] bf16, moving = Wg^T slab [128,512] bf16, all 4 strips stream
concurrently (~206ns per (chunk,half) group), outputs land as rows
[1,512] at PSUM partitions {0,32,64,96}. VectorE reads each full PSUM
bank fusing the +Ig add into a bf16 tile (dead lanes free), PE
transposes [128,128] bf16 blocks back to column layout (bf16 identity:
1 cyc/row), and per half a single batched [128,16] add + tanh updates
all 4 individuals' states (column order: col = 4*chunk + q), so the
next unit's first matvecs only wait on the first half's tanh.

Decode runs on the PE column strips per round (round 0's decode hides
in round 1's DMA tail); output is [128, 2]: partition 32q+a = action
dim a of individual q, column = round.
"""

import os
import sys
from contextlib import ExitStack

import numpy as np

for _p in ("/opt/trn_rl_repo", "/root/.axon_site/_ro/trn_rl_repo"):
    if os.path.isdir(_p) and _p not in sys.path:
        sys.path.append(_p)

import concourse.bass as bass  # noqa: E402
import concourse.tile as tile  # noqa: E402
from concourse import bacc, mybir  # noqa: E402
from concourse.bass_utils import run_bass_kernel_spmd  # noqa: E402

DT = 0.1
ITERS = int(1.0 // DT)  # == 9: reference.py uses `int(1.0 // DT)`, and 1.0//0.1 == 9.0
B_FULL, N, OBS, ADIM = 64, 1024, 64, 16
NCORES = 8
BPC = B_FULL // NCORES  # individuals per core
P = 128
NCH = 8                 # 128-chunks per vector
RQ = 4                  # individuals per round (one per PE column strip)
NR = 2                  # rounds
F32 = mybir.dt.float32
BF16 = mybir.dt.bfloat16


def make_pools(ctx, tc):
    return dict(
        const=ctx.enter_context(tc.tile_pool(name="const", bufs=1)),
        wp1=ctx.enter_context(tc.tile_pool(name="w1", bufs=BPC * NCH)),
        state=ctx.enter_context(tc.tile_pool(name="state", bufs=2)),
        prow=ctx.enter_context(tc.tile_pool(name="prow", bufs=6, space="PSUM")),
        ptr=ctx.enter_context(tc.tile_pool(name="ptr", bufs=2, space="PSUM")),
    )


# W tile groups: (start chunk, n chunks). Small first for a fast matmul
# start, large for DMA throughput; single queue keeps arrival order equal
# to consumption order.
WGROUPS = [(h, 1) for h in range(NCH)]


def load_w(nc, wpools, ins):
    """Load W as consumption-ordered tiles on the sync queue.

    Returns {(b, h): (tile, col offset of chunk h)}.
    """
    w_sb = {}
    for r in range(NR):
        for h0, nch in WGROUPS:
            for q in range(RQ):
                b = RQ * r + q
                wt = wpools[f"wp{nch}"].tile(
                    [P, nch * N], BF16, tag=f"w{nch}", name=f"w{b}g{h0}")
                nc.sync.dma_start(
                    wt[:], ins["Wsb"][b][:, h0 * N:(h0 + nch) * N])
                for h in range(h0, h0 + nch):
                    w_sb[b, h] = (wt, (h - h0) * N)
    return w_sb


def kernel_body(ctx, tc, ins, out_ap, iters=ITERS, pools=None, w_sb=None):
    nc = tc.nc
    Tanh = mybir.ActivationFunctionType.Tanh
    add = mybir.AluOpType.add
    mult = mybir.AluOpType.mult
    sub = mybir.AluOpType.subtract

    p = pools if pools is not None else make_pools(ctx, tc)
    const, state = p["const"], p["state"]
    prow, ptr = p["prow"], p["ptr"]

    # ---- constants; s0c rides ahead of W on the sync queue ----
    s0_sb = const.tile([P, NR * 32], F32, tag="s0", name="s0")
    nc.sync.dma_start(s0_sb[:], ins["s0c"][:])
    cm_sb = const.tile([P, NR * 32], F32, tag="cm", name="cm")
    nc.gpsimd.dma_start(cm_sb[:], ins["cmc"][:])
    ident_sb = const.tile([P, P], BF16, tag="ident", name="ident")
    nc.gpsimd.dma_start(ident_sb[:], ins["ident"][:])
    igp_sb = {}
    for r in range(NR):
        igp_sb[r] = const.tile([P, 2 * 512], BF16, tag=f"ig{r}", name=f"ig{r}")
        nc.gpsimd.dma_start(igp_sb[r][:], ins["igp"][r])

    # ---- initial state + host-computed y0 per round ----
    y0_sb = const.tile([P, NR * 32], BF16, tag="y0c", name="y0c")
    nc.sync.dma_start(y0_sb[:], ins["y0c"][:])
    s_cur = [s0_sb[:, 32 * r:32 * r + 32] for r in range(NR)]
    y_cur = [y0_sb[:, 32 * r:32 * r + 32] for r in range(NR)]

    # ---- W loads: consumption-ordered tiles on the sync queue ----
    if w_sb is None:
        w_sb = load_w(nc, p, ins)

    # ---- decode constants (needed late; tail of gpsimd queue) ----
    dgt_sb = const.tile([P, BPC * P], F32, tag="dgt", name="dgt")
    nc.gpsimd.dma_start(dgt_sb[:], ins["dgtc"][:])
    db0_sb = const.tile([P, NR], F32, tag="db0", name="db0")
    nc.gpsimd.dma_start(db0_sb[:], ins["db0"][:])
    act_sb = const.tile([P, NR], F32, tag="act", name="act")

    pd_tile = [None]

    def decode_round(r):
        # action strip-matmuls for round r; borrows a prow bank
        if pd_tile[0] is None:
            pd_tile[0] = prow.tile([P, 512], F32, tag="pr", name="pd")
        pd = pd_tile[0]
        for q in range(RQ):
            b = RQ * r + q
            for h in range(NCH):
                nc.tensor.matmul(
                    pd[32 * q:32 * q + ADIM, r:r + 1],
                    dgt_sb[:, P * b + ADIM * h: P * b + ADIM * h + ADIM],
                    s_cur[r][:, 4 * h + q: 4 * h + q + 1],
                    start=(h == 0), stop=(h == NCH - 1),
                    tile_position=(0, 32 * q),
                )

    # ---- recurrent loop: all of round 0's iterations, then round 1's.
    # Round r's W arrives first (single queue, round-major), so round 0
    # runs at full speed from ~1/2 DMA time while round 1's W streams in;
    # round 1 starts right when its W lands. State updates split in half
    # so the next unit's first matvecs never wait on the full tanh. ----
    for r in range(NR):
        for t in range(iters):
            tmp = state.tile([P, 32], F32, tag=f"t{r}", name=f"t{r}")
            nc.gpsimd.tensor_tensor(
                tmp[:], cm_sb[:, 32 * r:32 * r + 32], s_cur[r][:], op=mult)
            pr = [prow.tile([P, 512], F32, tag="pr", name="pr")
                  for _ in range(2)]
            for j in range(2):
                for h in range(NCH):
                    for q in range(RQ):
                        b = RQ * r + q
                        wt, woff = w_sb[b, h]
                        nc.tensor.matmul(
                            pr[j][32 * q:32 * q + 1, :],
                            y_cur[r][:, 4 * h + q: 4 * h + q + 1],
                            wt[:, woff + 512 * j:woff + 512 * j + 512],
                            start=(h == 0), stop=(h == NCH - 1),
                            tile_position=(0, 32 * q),
                        )
            u = [None, None]
            for j in range(2):
                u[j] = state.tile([P, 512], BF16, tag=f"u{r}{j}",
                                  name=f"u{r}{j}")
                nc.vector.tensor_tensor(
                    u[j][:], pr[j][:],
                    igp_sb[r][:, 512 * j:512 * j + 512], op=add)
            pt = ptr.tile([P, NCH, P], BF16, tag="pt", name="pt")
            s_n = state.tile([P, 32], F32, tag=f"s{r}", name=f"s{r}")
            y_n = (state.tile([P, 32], BF16, tag=f"y{r}", name=f"y{r}")
                   if t < iters - 1 else None)
            for half in range(2):
                for t8 in range(4 * half, 4 * half + 4):
                    nc.tensor.transpose(
                        pt[:, t8, :],
                        u[half][:, 128 * (t8 % 4):128 * (t8 % 4) + 128],
                        ident_sb[:],
                    )
                ha, hb = 16 * half, 16 * half + 16
                nc.vector.tensor_tensor(
                    s_n[:, ha:hb], tmp[:, ha:hb],
                    pt[:, 4 * half:4 * half + 4, 0:97:32], op=add)
                if y_n is not None:
                    nc.scalar.activation(
                        y_n[:, ha:hb], s_n[:, ha:hb], Tanh)
            s_cur[r] = s_n
            if y_n is not None:
                y_cur[r] = y_n
        decode_round(r)

    # ---- final: action = pd - D@bias ----
    nc.vector.tensor_tensor(act_sb[:], pd_tile[0][:, 0:NR], db0_sb[:], op=sub)
    nc.sync.dma_start(out_ap[:], act_sb[:])
    pd_tile[0] = None


def build_nc(iters=ITERS, reps=1, w_once=False):
    nc = bacc.Bacc(
        "TRN2", target_bir_lowering=False, debug=False, enable_asserts=False,
    )
    ins = {}
    for name, shape, dt in [
        ("Wsb", [BPC, P, NCH * N], BF16),
        ("s0c", [P, NR * 32], F32),
        ("y0c", [P, NR * 32], BF16),
        ("cmc", [P, NR * 32], F32),
        ("igp", [NR, P, 2 * 512], BF16),
        ("dgtc", [P, BPC * P], F32),
        ("db0", [P, NR], F32),
        ("ident", [P, P], BF16),
    ]:
        ins[name] = nc.dram_tensor(name, shape, dt, kind="ExternalInput").ap()
    out_ap = nc.dram_tensor("act", [P, NR], F32, kind="ExternalOutput").ap()

    with tile.TileContext(nc) as tc:
        with ExitStack() as ctx:
            pools = make_pools(ctx, tc)
            w_shared = None
            if w_once:
                nc0 = tc.nc
                w_shared = load_w(nc0, pools, ins)
            for _rep in range(reps):
                kernel_body(ctx, tc, ins, out_ap, iters, pools,
                            w_sb=w_shared)
    nc.compile()
    return nc


def prep_in_maps(obs, v0, tau, gain, bias, W, mask, E, D):
    f = np.float32
    obs, v0, tau, gain, bias, W, mask, E, D = [
        np.asarray(x, dtype=f) for x in (obs, v0, tau, gain, bias, W, mask, E, D)
    ]
    import ml_dtypes
    bf16 = ml_dtypes.bfloat16

    g = np.where(gain == 0.0, f(1e-6), gain)    # exact-rescaling guard
    am = (DT / tau) * mask                      # [64, N]
    cm = (1.0 - DT / tau) * mask
    I = np.einsum("bno,bo->bn", E, obs)         # [64, N]
    Ig = g * (am * I + bias * (1.0 - cm))
    s0 = g * (v0 + bias)
    Wg = W * (g * am)[:, :, None] * mask[:, None, :]
    # device layout: w[b][k, h*N + m] = Wg[b, m, 128h+k]
    wdev = np.ascontiguousarray(
        Wg.transpose(0, 2, 1).reshape(B_FULL, NCH, P, N).transpose(0, 2, 1, 3)
    ).reshape(B_FULL, P, NCH * N).astype(bf16)

    def cols(x):  # [64, N] -> [core, p, 32r + 4t + q]  (n = p + 128 t)
        xc = x.reshape(NCORES, NR, RQ, NCH, P)
        return np.ascontiguousarray(
            xc.transpose(0, 4, 1, 3, 2)).reshape(NCORES, P, NR * 32)

    s0c = cols(s0)
    y0c = cols(np.tanh(s0)).astype(bf16)
    cmc = cols(cm)
    # padded Ig rows: igp[core, r, 32q, 512j + m] = Ig[8core+4r+q, 512j + m]
    igp = np.zeros((NCORES, NR, P, 2 * 512), bf16)
    for r in range(NR):
        for j in range(2):
            for q in range(RQ):
                igp[:, r, 32 * q, 512 * j:512 * j + 512] = Ig.reshape(
                    NCORES, BPC, N)[:, RQ * r + q, 512 * j:512 * j + 512]
    Dg = D / g[:, None, :]
    dgt = np.ascontiguousarray(
        Dg.transpose(0, 2, 1).reshape(B_FULL, NCH, P, ADIM).transpose(0, 2, 1, 3)
    ).reshape(B_FULL, P, P)
    dgtc = np.ascontiguousarray(
        dgt.reshape(NCORES, BPC, P, P).transpose(0, 2, 1, 3)
    ).reshape(NCORES, P, BPC * P)
    db0 = np.einsum("ban,bn->ba", D, bias)      # [64, ADIM]
    # db0c[core][32q + a, r] = db0[8core + 4r + q, a]
    db0c = np.zeros((NCORES, P, NR), f)
    for r in range(NR):
        for q in range(RQ):
            db0c[:, 32 * q:32 * q + ADIM, r] = db0.reshape(
                NCORES, NR, RQ, ADIM)[:, r, q]
    ident = np.eye(P, dtype=f).astype(bf16)

    in_maps = []
    for core in range(NCORES):
        s = slice(core * BPC, (core + 1) * BPC)
        in_maps.append({
            "Wsb": np.ascontiguousarray(wdev[s]),
            "s0c": s0c[core], "y0c": y0c[core],
            "cmc": cmc[core], "igp": igp[core],
            "dgtc": dgtc[core], "db0": db0c[core], "ident": ident,
        })
    return in_maps


_NC_CACHE = None


def _get_nc():
    global _NC_CACHE
    if _NC_CACHE is None:
        _NC_CACHE = build_nc()
    return _NC_CACHE


def kernel(obs, v0, tau, gain, bias, W, mask, E, D):
    nc = _get_nc()
    in_maps = prep_in_maps(obs, v0, tau, gain, bias, W, mask, E, D)
    res = run_bass_kernel_spmd(nc, in_maps, core_ids=list(range(NCORES)))
    # device output is [128, NR] per core: [32q + a, r] = action[4r+q, a]
    out = np.empty((B_FULL, ADIM), np.float32)
    for core in range(NCORES):
        a = np.asarray(res.results[core]["act"])
        for r in range(NR):
            for q in range(RQ):
                out[core * BPC + RQ * r + q] = a[32 * q:32 * q + ADIM, r]
    return out
